# revision 1
# baseline (speedup 1.0000x reference)
"""Euclidean contrastive loss on 8 Trainium2 NeuronCores (Bass/Tile).

Strategy (SPMD, one program for all 8 cores, per-core data differs):
  - Host: cast tokens/labels to bf16; for core c inputs are rotated by c*1024
    rows so each core's "own" rows are rows 0..1023 of its copy -> all device
    slice offsets are compile-time constants.
  - Device per core:
      * prep: label one-hots / class counts (n_pos via a tiny PE matmul),
        row norms on ACT (Square + accum), rsqrt, normalize rows on DVE,
        bounce normalized bf16 rows to HBM, DMA-xbar transpose into
        tT[k] = [128, 8192] (4 K-tiles of the 512 feature dim).
      * sim = tT.T @ tT per (128-row block) x (2048-col group) in PSUM fp32;
        diagonal fix sim[ii] -= 2 (fused DVE op on the one diag 512-slice);
        dist/tau = ACT Sqrt(scale*sim + bias) -> fp16 dist tiles.
      * masked dist sums via PE: T[v, j] = sum_i onehot(label_i==v) dist[i, j]
        accumulated over the 4 blocks of a phase pair, then one fused DVE op
        (T * OHT + row-accum) -> per-chunk class partials (host sums them).
      * exp(-dist/tau) in-place per block ([128, 8192] single ACT op) with
        free row-sum accumulation -> LSE = Ln(rowsum) (one ACT op at end).
      * ACT instructions are dependency-chained in emission order so the
        scheduler cannot interleave sqrt/exp table sets (~5 table loads).
  - Host: loss = [sum(npos*LSE) + sum(ms partials) - 1024*8*(2/tau)] / sum(npos).
"""

import os
import sys

import numpy as np
import ml_dtypes

try:
    import concourse.bass as bass  # noqa: F401
except ImportError:  # harness runs from a bare directory
    for p in ("/opt/trn_rl_repo", os.path.expanduser("~/.axon_site/_ro/trn_rl_repo")):
        if os.path.isdir(p) and p not in sys.path:
            sys.path.insert(0, p)
    import concourse.bass as bass  # noqa: F401

import concourse.mybir as mybir
import concourse.tile as tile
from concourse import bacc, bass_utils
from concourse.tile import add_dep_helper

N, D, NCORES = 8192, 512, 8
RPC = N // NCORES        # 1024 rows per core
NB = RPC // 128          # 8 row blocks of 128
KT = D // 128            # 4 contraction tiles
GW = 2048                # column group width
NG = N // GW             # 4 column groups
NCH = N // 512           # 16 column chunks
PH = 2                   # phase pairs
BPP = NB // PH           # blocks per phase pair (4)
NCLS = 100               # label classes

BF16 = mybir.dt.bfloat16
FP16 = mybir.dt.float16
FP32 = mybir.dt.float32
AX = mybir.AxisListType.X
OP = mybir.AluOpType
AF = mybir.ActivationFunctionType

_CACHE: dict = {}
last_results = None  # test harness reads exec_time_ns from here


def _build(tau: float):
    nc = bacc.Bacc(
        "TRN2",
        target_bir_lowering=False,
        debug=False,
        enable_asserts=False,
        num_devices=NCORES,
    )
    tok = nc.dram_tensor("tok", [N, D], BF16, kind="ExternalInput")
    lab_bc = nc.dram_tensor("lab_bc", [128, N], BF16, kind="ExternalInput")
    lab_rows = nc.dram_tensor("lab_rows", [128, NB], FP32, kind="ExternalInput")
    out = nc.dram_tensor("part", [128, 2 * NB], FP32, kind="ExternalOutput")
    out2 = nc.dram_tensor("part2", [128, PH * NCH], FP32, kind="ExternalOutput")

    A = 2.0 / (tau * tau)  # (dist/tau)^2 = A - A*sim

    act_chain = []  # ACT instructions in required execution order

    def act(*args, **kwargs):
        inst = nc.scalar.activation(*args, **kwargs)
        act_chain.append(inst)
        return inst

    with tile.TileContext(nc) as tc:
        with (
            tc.tile_pool(name="persist", bufs=1) as pp,
            tc.tile_pool(name="rows", bufs=16) as rows,
            tc.tile_pool(name="dist", bufs=BPP) as distp,
            tc.tile_pool(name="scratch", bufs=1) as sc,
            tc.tile_pool(name="psum", bufs=2, space="PSUM") as psum,
            tc.tile_pool(name="dram", bufs=1, space="DRAM") as dram,
        ):
            # ---- persistent tiles ----
            tT = [
                pp.tile([128, N], BF16, tag=f"tT{k}", name=f"tT{k}")
                for k in range(KT)
            ]
            Lc = pp.tile([128, N], BF16, tag="Lc")
            OHT = pp.tile([128, N], BF16, tag="OHT")  # rows 0..99: class one-hot
            lr = pp.tile([128, NB], FP32, tag="lr")
            dms = pp.tile([128, 4 * 512], BF16, tag="dms")
            ohb = [
                pp.tile([128, NCLS], FP16, tag=f"ohb{m}", name=f"ohb{m}")
                for m in range(NB)
            ]
            cnts = pp.tile([128, 1], FP32, tag="cnts")
            cnts_bf = pp.tile([128, 1], BF16, tag="cnts_bf")
            norm2 = pp.tile([128, 64], FP32, tag="norm2")
            nrm = pp.tile([128, 64], FP32, tag="nrm")
            inv = pp.tile([128, 64], FP32, tag="inv")
            rowsum = pp.tile([128, NB], FP32, tag="rowsum")
            lse = pp.tile([128, NB], FP32, tag="lse")
            np2 = pp.tile([128, NB], FP32, tag="np2")
            msp = pp.tile([128, PH * NCH], FP32, tag="msp")
            outp = pp.tile([128, 2 * NB], FP32, tag="outp")
            biasA = pp.tile([128, 1], FP32, tag="biasA")

            norm_hbm = dram.tile([N, D], BF16)

            nc.gpsimd.memset(biasA[:], float(A))

            # ---- labels ----
            nc.sync.dma_start(Lc[:], lab_bc[:, :])
            nc.sync.dma_start(lr[:], lab_rows[:, :])

            # ---- index tiles ----
            # diag masks dm_k[p, f] = (f - p == 128k)
            iot = sc.tile([128, 512], mybir.dt.int32, tag="iot")
            nc.gpsimd.iota(iot[:], pattern=[[1, 512]], base=0, channel_multiplier=-1)
            iotf = sc.tile([128, 512], FP32, tag="iotf")
            nc.vector.tensor_copy(iotf[:], iot[:])
            for kk in range(4):
                nc.vector.tensor_scalar(
                    dms[:, kk * 512:(kk + 1) * 512], iotf[:],
                    float(kk * 128), None, op0=OP.is_equal,
                )
            # iotac[p, 0] = p ; iotrow[p, f] = f (f < NCLS)
            iotac = sc.tile([128, 1], mybir.dt.int32, tag="iotac")
            nc.gpsimd.iota(iotac[:], pattern=[[1, 1]], base=0, channel_multiplier=1)
            iotacf = sc.tile([128, 1], FP32, tag="iotacf")
            nc.vector.tensor_copy(iotacf[:], iotac[:])
            iotrow = sc.tile([128, NCLS], mybir.dt.int32, tag="iotrow")
            nc.gpsimd.iota(iotrow[:], pattern=[[1, NCLS]], base=0, channel_multiplier=0)
            iotrowf = sc.tile([128, NCLS], FP32, tag="iotrowf")
            nc.vector.tensor_copy(iotrowf[:], iotrow[:])

            # ---- class one-hots + counts + n_pos ----
            # OHT[v, j] = (label_j == v)
            nc.vector.tensor_scalar(
                OHT[0:NCLS, :], Lc[0:NCLS, :], iotacf[0:NCLS, :], None,
                op0=OP.is_equal,
            )
            nc.vector.reduce_sum(cnts[0:NCLS, :], OHT[0:NCLS, :], axis=AX)
            nc.vector.tensor_copy(cnts_bf[0:NCLS, :], cnts[0:NCLS, :])
            ohbt = sc.tile([128, 128], BF16, tag="ohbt")
            for m in range(NB):
                # ohb[m][i, v] = (label_{block m, row i} == v)  (lhsT for T-matmul)
                nc.vector.tensor_scalar(
                    ohb[m][:, :], iotrowf[:], lr[:, m:m + 1], None, op0=OP.is_equal,
                )
                # ohbt[v, i] = same, transposed layout (lhsT for n_pos matmul)
                nc.vector.tensor_scalar(
                    ohbt[0:NCLS, :], Lc[0:NCLS, m * 128:(m + 1) * 128],
                    iotacf[0:NCLS, :], None, op0=OP.is_equal,
                )
                npp = psum.tile([128, GW], FP32, tag="ps", name=f"npp{m}")
                nc.tensor.matmul(
                    npp[:, 0:1], ohbt[0:NCLS, :], cnts_bf[0:NCLS, :],
                )
                nc.vector.tensor_scalar(
                    np2[:, m:m + 1], npp[:, 0:1], -1.0, None, op0=OP.add,
                )

            # ---- load rows, norms (ACT), normalize (DVE), bounce to HBM ----
            junk = sc.tile([128, D], BF16, tag="junk")
            rowts = []
            for j in range(64):
                rowt = rows.tile([128, D], BF16, tag="rowt")
                rowts.append(rowt)
                nc.sync.dma_start(rowt[:], tok[j * 128:(j + 1) * 128, :])
                act(junk[:], rowt[:], AF.Square, accum_out=norm2[:, j:j + 1])
                if j % 8 == 7:
                    g8 = j // 8
                    s = slice(g8 * 8, g8 * 8 + 8)
                    act(nrm[:, s], norm2[:, s], AF.Sqrt)
                    nc.vector.reciprocal(inv[:, s], nrm[:, s])
                    for jj in range(g8 * 8, g8 * 8 + 8):
                        rt = rowts[jj]
                        nc.vector.tensor_scalar(
                            rt[:], rt[:], inv[:, jj:jj + 1], None, op0=OP.mult,
                        )
                        nc.sync.dma_start(
                            norm_hbm[jj * 128:(jj + 1) * 128, :], rt[:],
                        )
                if j % 16 == 15:
                    jg = j // 16
                    for k in range(KT):
                        nc.sync.dma_start(
                            tT[k][:, jg * GW:(jg + 1) * GW],
                            norm_hbm[jg * GW:(jg + 1) * GW, k * 128:(k + 1) * 128],
                            transpose=True,
                        )

            # ---- main compute ----
            for ph in range(PH):
                blocks = range(ph * BPP, (ph + 1) * BPP)
                dist_of = {}
                # phase A: matmuls + diag fix + sqrt -> dist (fp16)
                for m in blocks:
                    dist_m = distp.tile([128, N], FP16, tag="dist")
                    dist_of[m] = dist_m
                    for g in range(NG):
                        ps = psum.tile([128, GW], FP32, tag="ps")
                        for k in range(KT):
                            lhsT = tT[k][:, m * 128:(m + 1) * 128]
                            for n in range(GW // 512):
                                nc.tensor.matmul(
                                    ps[:, n * 512:(n + 1) * 512],
                                    lhsT,
                                    tT[k][:, g * GW + n * 512: g * GW + (n + 1) * 512],
                                    start=(k == 0),
                                    stop=(k == KT - 1),
                                )
                        if g == 0:
                            nd = m // 4  # diag chunk within group 0
                            dsl = slice(nd * 512, (nd + 1) * 512)
                            nc.vector.scalar_tensor_tensor(
                                out=ps[:, dsl],
                                in0=dms[:, (m % 4) * 512:(m % 4 + 1) * 512],
                                scalar=-2.0,
                                in1=ps[:, dsl],
                                op0=OP.mult, op1=OP.add,
                            )
                        gs = slice(g * GW, (g + 1) * GW)
                        act(dist_m[:, gs], ps[:], AF.Sqrt, bias=biasA[:],
                            scale=float(-A))
                # masked-dist class sums: T[v, j] over the pair's blocks (PE),
                # then fused (T * OHT) row-accum (DVE) -> per-chunk partials
                tjunk = sc.tile([128, 512], BF16, tag="tjunk")
                for jc in range(NCH):
                    tps = psum.tile([128, GW], FP32, tag="ps", name=f"tps{ph}_{jc}")
                    for mi, m in enumerate(blocks):
                        nc.tensor.matmul(
                            tps[0:NCLS, 0:512],
                            ohb[m][:, :],
                            dist_of[m][:, jc * 512:(jc + 1) * 512],
                            start=(mi == 0),
                            stop=(mi == BPP - 1),
                        )
                    nc.vector.scalar_tensor_tensor(
                        out=tjunk[0:NCLS, :], in0=tps[0:NCLS, 0:512], scalar=1.0,
                        in1=OHT[0:NCLS, jc * 512:(jc + 1) * 512],
                        op0=OP.mult, op1=OP.mult,
                        accum_out=msp[0:NCLS, ph * NCH + jc:ph * NCH + jc + 1],
                    )
                # phase B: exp in place, one op per block, rowsum via accum
                for m in blocks:
                    act(dist_of[m][:, :], dist_of[m][:, :], AF.Exp, scale=-1.0,
                        accum_out=rowsum[:, m:m + 1])

            # ---- LSE + finalize ----
            act(lse[:, :], rowsum[:, :], AF.Ln)
            for m in range(NB):
                nc.vector.scalar_tensor_tensor(
                    out=outp[:, m:m + 1], in0=np2[:, m:m + 1], scalar=1.0,
                    in1=lse[:, m:m + 1], op0=OP.mult, op1=OP.mult,
                )
            nc.vector.tensor_copy(outp[:, NB:2 * NB], np2[:, :])
            nc.sync.dma_start(out[:, :], outp[:])
            nc.sync.dma_start(out2[:, :], msp[:])

            # ---- pin ACT execution order (stop table-set thrash) ----
            for a, b in zip(act_chain, act_chain[1:]):
                add_dep_helper(b.ins, a.ins, reason="act table-set order")

    nc.compile()
    return nc


def _get_program(tau: float):
    if tau not in _CACHE:
        _CACHE[tau] = _build(tau)
    return _CACHE[tau]


def make_in_maps(tokens: np.ndarray, labels: np.ndarray):
    bf = ml_dtypes.bfloat16
    tok_bf = np.asarray(tokens, dtype=np.float32).astype(bf)
    lab_f = np.asarray(labels).astype(np.float32)
    in_maps = []
    for c in range(NCORES):
        sh = c * RPC
        tok_rot = np.roll(tok_bf, -sh, axis=0)
        lab_rot = np.roll(lab_f, -sh)
        lab_bc = np.ascontiguousarray(
            np.broadcast_to(lab_rot.astype(bf)[None, :], (128, N))
        )
        lab_rows = np.ascontiguousarray(
            lab_rot[:RPC].reshape(NB, 128).T.astype(np.float32)
        )
        in_maps.append({
            "tok": np.ascontiguousarray(tok_rot),
            "lab_bc": lab_bc,
            "lab_rows": lab_rows,
        })
    return in_maps


def _install_ntff_hook_shim():
    """Provide antenv.axon_hooks if the image lacks it (NTFF profiling via
    direct ctypes calls into libaxon_pjrt.so)."""
    try:
        from antenv.axon_hooks import get_axon_ntff_profile_hook  # noqa: F401
        return True
    except ImportError:
        pass
    so_path = "/opt/axon/libaxon_pjrt.so"
    if not os.path.exists(so_path):
        return False
    import contextlib
    import ctypes
    import types

    lib = ctypes.CDLL(so_path)
    if not hasattr(lib, "axon_start_nrt_profile"):
        return False
    lib.axon_start_nrt_profile.argtypes = [
        ctypes.POINTER(ctypes.c_int64), ctypes.c_size_t,
    ]
    lib.axon_start_nrt_profile.restype = ctypes.c_int64
    lib.axon_stop_nrt_profile.argtypes = [ctypes.c_char_p]
    lib.axon_stop_nrt_profile.restype = ctypes.c_int64

    @contextlib.contextmanager
    def _hook(output_dir, device_ids):
        import jax
        jax.devices()
        if device_ids:
            ids = (ctypes.c_int64 * len(device_ids))(*device_ids)
            rc = lib.axon_start_nrt_profile(ids, len(device_ids))
        else:
            rc = lib.axon_start_nrt_profile(None, 0)
        if rc != 0:
            raise RuntimeError(f"axon_start_nrt_profile rc={rc}")
        try:
            yield
        finally:
            n = lib.axon_stop_nrt_profile(str(output_dir).encode())
            if n < 0:
                raise RuntimeError(f"axon_stop_nrt_profile rc={n}")
            print(f"profile: {n} file(s) written to {output_dir}")

    mod = types.ModuleType("antenv.axon_hooks")
    mod.get_axon_ntff_profile_hook = lambda: _hook
    mod.set_axon_ntff_profile_hook = lambda h: None
    sys.modules["antenv.axon_hooks"] = mod
    return True


def kernel(tokens, labels, temperature=0.07):
    global last_results
    tau = float(temperature)
    nc = _get_program(tau)
    in_maps = make_in_maps(tokens, labels)
    trace = bool(int(os.environ.get("KBENCH_TRACE", "0")))
    if trace:
        trace = _install_ntff_hook_shim()
    res = bass_utils.run_bass_kernel_spmd(
        nc, in_maps, core_ids=list(range(NCORES)),
        trace=trace,
    )
    last_results = res
    num = 0.0
    den = 0.0
    for c in range(NCORES):
        p = res.results[c]["part"]
        p2 = res.results[c]["part2"]
        num += p[:, :NB].astype(np.float64).sum()          # sum npos*LSE
        num += p2[:NCLS, :].astype(np.float64).sum()       # sum mask*dist/tau
        num -= RPC * (2.0 / tau)                           # diag correction
        den += p[:, NB:].astype(np.float64).sum()
    return np.float32(num / den)

